# revision 1
# baseline (speedup 1.0000x reference)
"""Multi-head attention (B=4, S=2048, D=1024, H=16) on 8 Trainium2 NeuronCores.

Sharding: 4-way data-parallel over batch x 2-way tensor-parallel over heads
(Megatron-style).  Core c handles batch c//2 and head-group c%2 (8 of 16
heads).  Each core computes qkv for its 512 q/k/v channels, attention for its
8 heads, and a row-parallel partial projection [S, D].  The host sums the two
partial outputs per batch and adds b_proj.

Per-core kernel strategy:
  - x^T tiles produced on-chip via PE transpose (DMA transpose unsupported
    for fp32).
  - Q^T, K^T computed channel-major [ch, s] from w-chunks (lhsT) x x^T (rhs);
    V computed natural [s, ch] from x^T-chunks (lhsT) x w_v (rhs).
  - Scores computed transposed: S^T[kj, qi] = K Q^T so softmax normalization
    folds into the PV matmul: lhsT = [V | ones] yields attn^T[ch, qi] plus
    row-sums in one accumulated matmul chain (one PSUM start per bank).
  - All big matmuls in float32r (TF32-like, full PE rate at N=512, rms err
    ~1.5e-4); operands rounded via compute-engine copies as walrus requires.
  - exp on ScalarE with the 1/sqrt(hd) scale folded in; normalization via
    a K=1 PE broadcast of the row-sums + approx-reciprocal + multiply at
    PSUM eviction, deferred one block so it never stalls the QK->exp chain.
"""

import sys
from contextlib import ExitStack

for _p in ("/opt/trn_rl_repo", "/root/.axon_site/_ro/trn_rl_repo"):
    if _p not in sys.path:
        sys.path.insert(0, _p)

import numpy as np

import concourse.bass as bass  # noqa: F401
import concourse.mybir as mybir
import concourse.tile as tile
from concourse import bacc
from concourse.bass_utils import run_bass_kernel_spmd
from concourse.masks import make_identity

F32 = mybir.dt.float32
F32R = mybir.dt.float32r
EXP = mybir.ActivationFunctionType.Exp

N_CORES = 8
FULL_B, FULL_S, FULL_D, FULL_H = 4, 2048, 1024, 16
HEAD_DIM = 64


def build_core_program(S=FULL_S, D=FULL_D, HL=FULL_H // 2, hd=HEAD_DIM):
    """Build the single-core Bass program (runs SPMD on all 8 cores with
    per-core input shards)."""
    CH = HL * hd            # local q (= k = v) channels (512)
    DC = D // 128           # d-chunks (qkv contraction)
    CCQ = CH // 128         # ch-chunks for Q (and K)
    CCA = CH // 128         # ch-chunks of attn output (proj contraction)
    SC = S // 128           # 128-row s-chunks (also kj-chunks)
    SBLK = S // 512         # 512-row s-blocks in the qkv phase
    QBS = min(1024, S)      # qi block size in the attention phase
    QB = S // QBS
    NB = QBS // 512
    scale = float(hd) ** -0.5

    nc = bacc.Bacc("TRN2", target_bir_lowering=False, debug=False,
                   num_devices=N_CORES)

    x_ap = nc.dram_tensor("x", [S, D], F32, kind="ExternalInput").ap()
    wqkv_ap = nc.dram_tensor("w_qkv", [D, 3 * CH], F32, kind="ExternalInput").ap()
    bqkv_ap = nc.dram_tensor("b_qkv", [3 * CH], F32, kind="ExternalInput").ap()
    wproj_ap = nc.dram_tensor("w_proj", [CH, D], F32, kind="ExternalInput").ap()
    out_ap = nc.dram_tensor("out", [S, D], F32, kind="ExternalOutput").ap()

    with tile.TileContext(nc) as tc, ExitStack() as es:
        constp = es.enter_context(tc.tile_pool(name="const", bufs=1))
        qk_es = es.enter_context(ExitStack())
        qkp = qk_es.enter_context(tc.tile_pool(name="qk", bufs=1))

        ident = constp.tile([128, 128], F32)
        make_identity(nc, ident)
        bias_qk = constp.tile([128, 2 * CCQ], F32)
        nc.sync.dma_start(bias_qk[:],
                          bqkv_ap[0:2 * CH].rearrange("(c p) -> p c", p=128))
        bv_row = constp.tile([1, CH], F32)
        nc.sync.dma_start(bv_row[:],
                          bqkv_ap[2 * CH:3 * CH].rearrange("(a b) -> a b", a=1))
        bv_bc = constp.tile([128, CH], F32)
        nc.gpsimd.partition_broadcast(bv_bc[:], bv_row[0:1, :])
        ones_f = constp.tile([128, SC * HL], F32)
        nc.vector.memset(ones_f[:], 1.0)
        ones65_f = constp.tile([65, 64], F32)
        nc.vector.memset(ones65_f[:], 1.0)
        ones65 = constp.tile([65, 64], F32R)
        nc.vector.tensor_copy(ones65[:], ones65_f[:])

        # persistent activations (released after phase 2)
        qT = qkp.tile([128, CCQ, S], F32R)            # [ch, s]
        kT = qkp.tile([128, CCQ, S], F32R)
        vp = qkp.tile([128, SC, HL, hd + 2], F32R)    # [s | kj, head, V|1|pad]
        nc.vector.tensor_copy(vp[:, :, :, hd],
                              ones_f[:].rearrange("p (a b) -> p a b", b=HL))

        # ---------------- phase 1: weights, x^T, qkv ----------------
        with ExitStack() as p1:
            wqp = p1.enter_context(tc.tile_pool(name="wq", bufs=1))
            stagep = p1.enter_context(tc.tile_pool(name="stage", bufs=1))
            ps1 = p1.enter_context(tc.tile_pool(name="ps1", bufs=1, space="PSUM"))

            wq_r = wqp.tile([128, DC, 3 * CH], F32R)
            HW_ = 3 * CH // 2
            for dc in range(DC):
                for half in range(2):
                    wq_f = wqp.tile([128, HW_], F32, tag="wq_st", bufs=2)
                    nc.sync.dma_start(
                        wq_f[:], wqkv_ap[dc * 128:(dc + 1) * 128,
                                         half * HW_:(half + 1) * HW_])
                    nc.vector.tensor_copy(
                        wq_r[:, dc, half * HW_:(half + 1) * HW_], wq_f[:])

            for sb_i in range(SBLK):
                xs_tiles = []
                for i in range(4):
                    xsi = stagep.tile([128, D], F32, tag="xs", bufs=4)
                    nc.sync.dma_start(
                        xsi[:], x_ap[sb_i * 512 + i * 128:
                                     sb_i * 512 + (i + 1) * 128, :])
                    xs_tiles.append(xsi)
                xT = stagep.tile([128, DC, 512], F32R, tag="xT", bufs=2)
                for dc in range(DC):
                    tp = ps1.tile([128, 512], F32, tag="tp", bufs=3)
                    for i in range(4):
                        nc.tensor.transpose(tp[:, i * 128:(i + 1) * 128],
                                            xs_tiles[i][:, dc * 128:(dc + 1) * 128],
                                            ident[:])
                    nc.vector.tensor_copy(xT[:, dc, :], tp[:])
                # Q^T / K^T chunks: out [ch 128, s 512]
                for cc in range(2 * CCQ):
                    qp = ps1.tile([128, 512], F32, tag="qkv", bufs=4)
                    for dc in range(DC):
                        nc.tensor.matmul(qp[:],
                                         wq_r[:, dc, cc * 128:(cc + 1) * 128],
                                         xT[:, dc, :],
                                         start=(dc == 0), stop=(dc == DC - 1))
                    dst = qT if cc < CCQ else kT
                    cc_l = cc if cc < CCQ else cc - CCQ
                    nc.vector.tensor_scalar_add(
                        dst[:, cc_l, sb_i * 512:(sb_i + 1) * 512], qp[:],
                        bias_qk[:, cc:cc + 1])
                # V natural: out [s 128, ch 512]
                for si in range(4):
                    vps = ps1.tile([128, CH], F32, tag="qkv", bufs=4)
                    for dc in range(DC):
                        nc.tensor.matmul(vps[:],
                                         xT[:, dc, si * 128:(si + 1) * 128],
                                         wq_r[:, dc, 2 * CH:3 * CH],
                                         start=(dc == 0), stop=(dc == DC - 1))
                    sc_g = sb_i * 4 + si
                    nc.vector.tensor_add(
                        vp[:, sc_g, :, 0:hd],
                        vps[:].rearrange("p (h e) -> p h e", e=hd),
                        bv_bc[:].rearrange("p (h e) -> p h e", e=hd))

        # ---------------- phase 2: attention ----------------
        attn_es = es.enter_context(ExitStack())
        attnp = attn_es.enter_context(
            tc.tile_pool(name="attn", bufs=1, side="right"))
        attn_r = attnp.tile([128, CCA, S], F32R)      # normalized attn^T (f32r)
        attn_t_odd = attnp.tile([128, CCA, S], F32)   # staging for odd heads

        with ExitStack() as p2:
            workp = p2.enter_context(tc.tile_pool(name="w2", bufs=1))
            ps2 = p2.enter_context(tc.tile_pool(name="ps2", bufs=1, space="PSUM"))

            def emit_norm(cc, base, qb, attn_ps):
                # normalize columns by 1/rowsum, store into attn_r.  Sums sit
                # on psum row 64; PE-broadcast to rows 0..63 (K=1 matmul),
                # approx-reciprocal, multiply.  Emitted one block late so the
                # PE never stalls the QK->exp chain on this detour.
                sums_sb = workp.tile([65, QBS], F32R, tag="asb", bufs=2)
                nc.vector.tensor_copy(sums_sb[64:65, :], attn_ps[64:65, :])
                bc = ps2.tile([64, QBS], F32, tag="sc", bufs=2)
                for nb in range(NB):
                    nc.tensor.matmul(bc[:, nb * 512:(nb + 1) * 512],
                                     ones65[64:65, 0:64],
                                     sums_sb[64:65, nb * 512:(nb + 1) * 512],
                                     start=True, stop=True,
                                     tile_position=(64, 0))
                recip = workp.tile([64, QBS], F32, tag="norm", bufs=2)
                nc.vector.reciprocal_approx_fast(recip[:], bc[:])
                if base == 0:
                    nc.vector.tensor_mul(
                        attn_r[0:64, cc, qb * QBS:(qb + 1) * QBS],
                        attn_ps[0:64, :], recip[:])
                else:
                    asb = workp.tile([64, QBS], F32, tag="asb", bufs=2)
                    nc.vector.tensor_mul(asb[:], attn_ps[0:64, :], recip[:])
                    nc.sync.dma_start(
                        attn_t_odd[64:128, cc, qb * QBS:(qb + 1) * QBS],
                        asb[:])
                    if qb == QB - 1:
                        nc.vector.tensor_copy(attn_r[64:128, cc, :],
                                              attn_t_odd[64:128, cc, :])

            pending = None
            for h in range(HL):
                cc, base = h // 2, (h % 2) * 64
                for qb in range(QB):
                    attn_ps = ps2.tile([65, QBS], F32, tag="attn", bufs=2)
                    for kj in range(SC):
                        if kj == 1 and pending is not None:
                            emit_norm(*pending)
                            pending = None
                        sc_ps = ps2.tile([128, QBS], F32, tag="sc", bufs=2)
                        for nb in range(NB):
                            nc.tensor.matmul(
                                sc_ps[:, nb * 512:(nb + 1) * 512],
                                kT[base:base + 64, cc, kj * 128:(kj + 1) * 128],
                                qT[base:base + 64, cc,
                                   qb * QBS + nb * 512:qb * QBS + (nb + 1) * 512],
                                start=True, stop=True,
                                tile_position=(base, 0))
                        pt = workp.tile([128, QBS], F32R, tag="pt", bufs=2)
                        nc.scalar.activation(pt[:], sc_ps[:], EXP, scale=scale)
                        for nb in range(NB):
                            nc.tensor.matmul(
                                attn_ps[:, nb * 512:(nb + 1) * 512],
                                vp[:, kj, h, 0:hd + 1],
                                pt[:, nb * 512:(nb + 1) * 512],
                                start=(kj == 0), stop=(kj == SC - 1))
                    pending = (cc, base, qb, attn_ps)
            emit_norm(*pending)

        qk_es.close()  # free qT/kT/vp

        # ---------------- phase 3: projection ----------------
        with ExitStack() as p3:
            w3 = p3.enter_context(tc.tile_pool(name="w3", bufs=1))
            ps3 = p3.enter_context(tc.tile_pool(name="ps3", bufs=1, space="PSUM"))


            wp_r = w3.tile([128, CCA, D], F32R)
            for cc4 in range(CCA):
                wp_f = w3.tile([128, D], F32, tag="wp_st", bufs=2)
                nc.sync.dma_start(wp_f[:],
                                  wproj_ap[cc4 * 128:(cc4 + 1) * 128, :])
                nc.vector.tensor_copy(wp_r[:, cc4, :], wp_f[:])
            for sc_i in range(SC):
                pp = ps3.tile([128, D], F32, tag="proj", bufs=3)
                for cc4 in range(CCA):
                    for nh in range(D // 512):
                        nc.tensor.matmul(
                            pp[:, nh * 512:(nh + 1) * 512],
                            attn_r[:, cc4, sc_i * 128:(sc_i + 1) * 128],
                            wp_r[:, cc4, nh * 512:(nh + 1) * 512],
                            start=(cc4 == 0), stop=(cc4 == CCA - 1))
                osb = w3.tile([128, D], F32, tag="osb", bufs=3)
                nc.vector.tensor_copy(osb[:], pp[:])
                nc.sync.dma_start(out_ap[sc_i * 128:(sc_i + 1) * 128, :], osb[:])

    nc.compile()
    return nc


def shard_inputs(x, w_qkv, b_qkv, w_proj):
    """Full inputs -> per-core input maps. Core c: batch c//2, head-group c%2."""
    B, S, D = x.shape
    CH = D // 2
    in_maps = []
    for c in range(N_CORES):
        b, g = c // 2, c % 2
        sl = slice(g * CH, (g + 1) * CH)
        w_s = np.concatenate(
            [w_qkv[:, 0 * D + g * CH:0 * D + (g + 1) * CH],
             w_qkv[:, 1 * D + g * CH:1 * D + (g + 1) * CH],
             w_qkv[:, 2 * D + g * CH:2 * D + (g + 1) * CH]], axis=1)
        b_s = np.concatenate(
            [b_qkv[0 * D + g * CH:0 * D + (g + 1) * CH],
             b_qkv[1 * D + g * CH:1 * D + (g + 1) * CH],
             b_qkv[2 * D + g * CH:2 * D + (g + 1) * CH]], axis=0)
        in_maps.append({
            "x": np.ascontiguousarray(x[b]),
            "w_qkv": np.ascontiguousarray(w_s),
            "b_qkv": np.ascontiguousarray(b_s),
            "w_proj": np.ascontiguousarray(w_proj[sl, :]),
        })
    return in_maps


_PROGRAM = None


def _get_program():
    global _PROGRAM
    if _PROGRAM is None:
        _PROGRAM = build_core_program()
    return _PROGRAM


def run_sharded(nc, in_maps, **kw):
    """run_bass_kernel_spmd with retries: the first execution on a freshly
    attached device occasionally dies with NRT_EXEC_UNIT_UNRECOVERABLE."""
    last = None
    for _ in range(3):
        try:
            return run_bass_kernel_spmd(nc, in_maps,
                                        core_ids=list(range(N_CORES)), **kw)
        except Exception as e:  # noqa: BLE001
            last = e
    raise last


def kernel(x, w_qkv, b_qkv, w_proj, b_proj):
    x = np.asarray(x, dtype=np.float32)
    w_qkv = np.asarray(w_qkv, dtype=np.float32)
    b_qkv = np.asarray(b_qkv, dtype=np.float32)
    w_proj = np.asarray(w_proj, dtype=np.float32)
    b_proj = np.asarray(b_proj, dtype=np.float32)

    nc = _get_program()
    in_maps = shard_inputs(x, w_qkv, b_qkv, w_proj)
    res = run_sharded(nc, in_maps)

    B, S, D = x.shape
    out = np.empty((B, S, D), dtype=np.float32)
    for b in range(B):
        out[b] = res.results[2 * b]["out"] + res.results[2 * b + 1]["out"] + b_proj
    return out



# revision 7
# speedup vs baseline: 2.2909x; 2.2909x over previous
"""Multi-head attention (B=4, S=2048, D=1024, H=16) on 8 Trainium2 NeuronCores.

Sharding: 4-way data-parallel over batch x 2-way tensor-parallel over heads
(Megatron-style).  Core c handles batch c//2 and head-group c%2 (8 of 16
heads).  Each core computes qkv for its 512 q/k/v channels, attention for its
8 heads, and a row-parallel partial projection [S, D].  The host sums the two
partial outputs per batch and adds b_proj.

v2 strategy (all-bf16, ScalarE-exp-bound pipeline):
  - Host pre-transposes x to x^T and casts all operands to bf16 (rel-err
    budget 2e-2 leaves ~30x margin; bf16 matmul runs at full PE rate and
    needs no on-device f32r rounding copies or PE transposes).
  - Q^T/K^T computed channel-major [ch, s] (w stationary, x^T moving);
    V natural [s, ch] (x^T chunks stationary, w_v moving) with a ones
    column appended per head for softmax row-sums.
  - Attention per (head-pair, 512-q block): the two heads of a ch-chunk sit
    in partition halves 0:64 / 64:128, so their QK matmuls use PE row groups
    (0,0)/(64,0) and overlap on HW.  One exp per kj covers both heads
    (N=1024 from PSUM).  PV is emitted one kj behind QK/exp so the PE FIFO
    never head-blocks the ScalarE exp stream (the kernel is exp-bound).
  - attn accumulators (with row-sums on partition 64 via the ones column)
    are evicted PSUM->SBUF by GpSimd immediately so the 2 attn banks
    recycle; softmax normalization (approx-reciprocal + partition-broadcast
    + mul) runs off the critical path on DVE/GpSimd; odd heads reach
    partitions 64:128 of the proj lhsT via a small SBUF->SBUF DMA (compute
    engines cannot shift partitions).
  - Q for later q-blocks and the projection of the previous q-block are
    interleaved into the attention stream as fillers under the exp window.
"""

import sys
from contextlib import ExitStack

for _p in ("/opt/trn_rl_repo", "/root/.axon_site/_ro/trn_rl_repo"):
    if _p not in sys.path:
        sys.path.insert(0, _p)

import numpy as np

import concourse.bass as bass  # noqa: F401
import concourse.mybir as mybir
import concourse.tile as tile
from concourse import bacc
from concourse.bass_utils import run_bass_kernel_spmd

F32 = mybir.dt.float32
BF16 = mybir.dt.bfloat16
EXP = mybir.ActivationFunctionType.Exp
BF16_NP = mybir.dt.np(BF16)

N_CORES = 8
FULL_B, FULL_S, FULL_D, FULL_H = 4, 2048, 1024, 16
HEAD_DIM = 64


def build_core_program(S=FULL_S, D=FULL_D, HL=FULL_H // 2, hd=HEAD_DIM):
    CH = HL * hd            # local q (= k = v) channels (512)
    DC = D // 128           # d-chunks (qkv contraction)
    CCQ = CH // 128         # 128-ch chunks of q/k = head pairs (4)
    SC = S // 128           # 128-row s-chunks (kj)
    QBS = 512               # q block size in attention
    QB = S // QBS
    scale = float(hd) ** -0.5

    nc = bacc.Bacc("TRN2", target_bir_lowering=False, debug=False,
                   num_devices=N_CORES)

    xT_ap = nc.dram_tensor("xT", [D, S], BF16, kind="ExternalInput").ap()
    wqk_ap = nc.dram_tensor("w_qk", [D, 2 * CH], BF16, kind="ExternalInput").ap()
    wv_ap = nc.dram_tensor("w_v", [D, CH], BF16, kind="ExternalInput").ap()
    wp_ap = nc.dram_tensor("w_proj", [CH, D], BF16, kind="ExternalInput").ap()
    bqk_ap = nc.dram_tensor("b_qk", [2 * CH], F32, kind="ExternalInput").ap()
    bv_ap = nc.dram_tensor("b_v", [CH], F32, kind="ExternalInput").ap()
    out_ap = nc.dram_tensor("out", [S, D], F32, kind="ExternalOutput").ap()

    with tile.TileContext(nc) as tc, ExitStack() as es:
        constp = es.enter_context(tc.tile_pool(name="const", bufs=1))
        actp = es.enter_context(tc.tile_pool(name="acts", bufs=1))
        workp = es.enter_context(tc.tile_pool(name="work", bufs=1, side="right"))
        ps_gen = es.enter_context(tc.tile_pool(name="psg", bufs=1, space="PSUM"))
        ps_att = es.enter_context(tc.tile_pool(name="psa", bufs=1, space="PSUM"))

        # ---- constants / weights / x^T loads (all bf16, no conversion) ----
        bias_qk = constp.tile([128, 2 * CCQ], F32)
        nc.sync.dma_start(bias_qk[:], bqk_ap.rearrange("(c p) -> p c", p=128))
        bv_row = constp.tile([1, CH], F32)
        nc.sync.dma_start(bv_row[:], bv_ap.rearrange("(a b) -> a b", a=1))
        bv_bc = constp.tile([128, CH], F32)
        nc.gpsimd.partition_broadcast(bv_bc[:], bv_row[0:1, :])
        warm = constp.tile([1, 16], F32)
        nc.vector.memset(warm[:], 0.0)
        # pull the exp table-load off the critical path
        nc.scalar.activation(warm[:], warm[:], EXP)

        wqk_r = constp.tile([128, DC, 2 * CH], BF16)
        wv_r = constp.tile([128, DC, CH], BF16)
        xT = constp.tile([128, DC, S], BF16)
        wp_r = constp.tile([128, CCQ, D], BF16)
        for dc in range(DC):
            nc.sync.dma_start(wqk_r[:, dc, :], wqk_ap[dc * 128:(dc + 1) * 128, :])
            nc.sync.dma_start(xT[:, dc, :], xT_ap[dc * 128:(dc + 1) * 128, :])
            nc.sync.dma_start(wv_r[:, dc, :], wv_ap[dc * 128:(dc + 1) * 128, :])
        for cc in range(CCQ):
            nc.sync.dma_start(wp_r[:, cc, :], wp_ap[cc * 128:(cc + 1) * 128, :])

        # ---- persistent activations ----
        qT = actp.tile([128, CCQ, S], BF16)           # [ch, s]
        kT = actp.tile([128, CCQ, S], BF16)
        vp = actp.tile([128, SC, HL, hd + 2], BF16)   # [s|kj, head, V|1|pad]
        nc.vector.memset(vp[:, :, :, hd], 1.0)
        attn_r = actp.tile([128, CCQ, S], BF16)       # normalized attn^T

        # ---- generation chain-groups (lead-in + fillers) ----
        def gen_qk(cc, sb, dst):
            """one [128ch, 512s] chunk of Q^T (dst=0) or K^T (dst=1)."""
            gp = ps_gen.tile([128, 512], F32, tag="gen", bufs=2)
            for dc in range(DC):
                nc.tensor.matmul(gp[:],
                                 wqk_r[:, dc, dst * CH + cc * 128:
                                       dst * CH + (cc + 1) * 128],
                                 xT[:, dc, sb * 512:(sb + 1) * 512],
                                 start=(dc == 0), stop=(dc == DC - 1))
            tgt = qT if dst == 0 else kT
            nc.vector.tensor_scalar_add(
                tgt[:, cc, sb * 512:(sb + 1) * 512], gp[:],
                bias_qk[:, dst * CCQ + cc:dst * CCQ + cc + 1])

        def gen_v(sc):
            """one [128s, 512ch] chunk of V (natural), bias added."""
            gp = ps_gen.tile([128, CH], F32, tag="gen", bufs=2)
            for dc in range(DC):
                nc.tensor.matmul(gp[:],
                                 xT[:, dc, sc * 128:(sc + 1) * 128],
                                 wv_r[:, dc, :],
                                 start=(dc == 0), stop=(dc == DC - 1))
            nc.vector.tensor_add(
                vp[:, sc, :, 0:hd],
                gp[:].rearrange("p (h e) -> p h e", e=hd),
                bv_bc[:].rearrange("p (h e) -> p h e", e=hd))

        def gen_proj(qb, sc_i):
            """projection for 128 q rows of block qb (both 512-d halves)."""
            sc_g = qb * (QBS // 128) + sc_i
            for dh in range(2):
                pp = ps_gen.tile([128, 512], F32, tag="gen", bufs=2)
                for cc in range(CCQ):
                    nc.tensor.matmul(pp[:],
                                     attn_r[:, cc, sc_g * 128:(sc_g + 1) * 128],
                                     wp_r[:, cc, dh * 512:(dh + 1) * 512],
                                     start=(cc == 0), stop=(cc == CCQ - 1))
                osb = workp.tile([128, 512], F32, tag="osb", bufs=3)
                nc.vector.tensor_copy(osb[:], pp[:])
                nc.sync.dma_start(
                    out_ap[sc_g * 128:(sc_g + 1) * 128,
                           dh * 512:(dh + 1) * 512], osb[:])

        # ---- lead-in: K (all s), V (all s), Q(qb0) ----
        for sb in range(S // 512):
            for cc in range(CCQ):
                gen_qk(cc, sb, 1)
        for sc in range(SC):
            gen_v(sc)
        for cc in range(CCQ):
            gen_qk(cc, 0, 0)

        # filler queue, consumed inside the attention stream
        fillers = []
        for sb in range(1, QB):
            for cc in range(CCQ):
                fillers.append((gen_qk, (cc, sb, 0)))
        f_i = [0]

        def pump():
            if f_i[0] < len(fillers):
                fn, args = fillers[f_i[0]]
                f_i[0] += 1
                fn(*args)

        def pv_pair(pt_t, kj, attnA, attnB, cc):
            nc.tensor.matmul(attnA[:], vp[:, kj, 2 * cc, 0:hd + 1],
                             pt_t[:, 0, :],
                             start=(kj == 0), stop=(kj == SC - 1))
            nc.tensor.matmul(attnB[:], vp[:, kj, 2 * cc + 1, 0:hd + 1],
                             pt_t[:, 1, :],
                             start=(kj == 0), stop=(kj == SC - 1))

        def emit_norm(cc, qb, attn_sbA, attn_sbB):
            """softmax-normalize both heads of a finished (pair, qb) block.
            Row sums live on partition 64 of the evicted accumulators; a tiny
            SBUF->SBUF DMA moves them to partition 0 (engines cannot)."""
            sums0 = workp.tile([1, 2 * QBS], F32, tag="sums0", bufs=2)
            nc.sync.dma_start(sums0[0:1, 0:QBS], attn_sbA[64:65, :])
            nc.sync.dma_start(sums0[0:1, QBS:2 * QBS], attn_sbB[64:65, :])
            rec = workp.tile([1, 2 * QBS], F32, tag="rec", bufs=2)
            nc.vector.reciprocal_approx_fast(rec[:], sums0[:])
            rbc = workp.tile([64, 2 * QBS], F32, tag="rbc", bufs=2)
            nc.gpsimd.partition_broadcast(rbc[:], rec[0:1, :])
            nc.vector.tensor_mul(
                attn_r[0:64, cc, qb * QBS:(qb + 1) * QBS],
                attn_sbA[0:64, :], rbc[:, 0:QBS])
            stg = workp.tile([64, QBS], BF16, tag="stg", bufs=2)
            nc.vector.tensor_mul(stg[:], attn_sbB[0:64, :],
                                 rbc[:, QBS:2 * QBS])
            nc.sync.dma_start(
                attn_r[64:128, cc, qb * QBS:(qb + 1) * QBS], stg[:])

        # ---- attention (exp-bound steady state) ----
        for qb in range(QB):
            for cc in range(CCQ):
                attnA = ps_att.tile([65, QBS], F32, tag="attnA", bufs=1)
                attnB = ps_att.tile([65, QBS], F32, tag="attnB", bufs=1)
                prev = None
                for kj in range(SC):
                    sc_t = ps_att.tile([128, 2, QBS], F32, tag="sc", bufs=2)
                    nc.tensor.matmul(
                        sc_t[:, 0, :],
                        kT[0:64, cc, kj * 128:(kj + 1) * 128],
                        qT[0:64, cc, qb * QBS:(qb + 1) * QBS],
                        start=True, stop=True, tile_position=(0, 0))
                    nc.tensor.matmul(
                        sc_t[:, 1, :],
                        kT[64:128, cc, kj * 128:(kj + 1) * 128],
                        qT[64:128, cc, qb * QBS:(qb + 1) * QBS],
                        start=True, stop=True, tile_position=(64, 0))
                    pt_t = workp.tile([128, 2, QBS], BF16, tag="pt", bufs=4)
                    nc.scalar.activation(pt_t[:], sc_t[:], EXP, scale=scale)
                    if prev is not None:
                        pv_pair(prev[1], prev[0], attnA, attnB, cc)
                    if kj % 4 == 1:
                        pump()
                    prev = (kj, pt_t)
                pv_pair(prev[1], prev[0], attnA, attnB, cc)
                # evict accumulators so the 2 attn banks recycle fast
                attn_sbA = workp.tile([65, QBS], F32, tag="asbA", bufs=2)
                attn_sbB = workp.tile([65, QBS], F32, tag="asbB", bufs=2)
                nc.vector.tensor_copy(attn_sbA[:], attnA[:])
                nc.vector.tensor_copy(attn_sbB[:], attnB[:])
                emit_norm(cc, qb, attn_sbA, attn_sbB)
            if qb > 0:
                for sc_i in range(QBS // 128):
                    fillers.append((gen_proj, (qb - 1, sc_i)))
        while f_i[0] < len(fillers):
            pump()
        for sc_i in range(QBS // 128):
            gen_proj(QB - 1, sc_i)

    nc.compile()
    return nc


def shard_inputs(x, w_qkv, b_qkv, w_proj):
    """Full inputs -> per-core input maps (host does transpose + bf16 cast).
    Core c: batch c//2, head-group c%2."""
    B, S, D = x.shape
    CH = D // 2
    in_maps = []
    for c in range(N_CORES):
        b, g = c // 2, c % 2
        sl = slice(g * CH, (g + 1) * CH)
        w_qk = np.concatenate(
            [w_qkv[:, 0 * D + g * CH:0 * D + (g + 1) * CH],
             w_qkv[:, 1 * D + g * CH:1 * D + (g + 1) * CH]],
            axis=1).astype(BF16_NP)
        w_v = np.ascontiguousarray(
            w_qkv[:, 2 * D + g * CH:2 * D + (g + 1) * CH]).astype(BF16_NP)
        b_qk = np.concatenate(
            [b_qkv[0 * D + g * CH:0 * D + (g + 1) * CH],
             b_qkv[1 * D + g * CH:1 * D + (g + 1) * CH]], axis=0)
        in_maps.append({
            "xT": np.ascontiguousarray(x[b].T).astype(BF16_NP),
            "w_qk": w_qk,
            "w_v": w_v,
            "w_proj": np.ascontiguousarray(w_proj[sl, :]).astype(BF16_NP),
            "b_qk": np.ascontiguousarray(b_qk),
            "b_v": np.ascontiguousarray(b_qkv[2 * D + g * CH:
                                              2 * D + (g + 1) * CH]),
        })
    return in_maps


_PROGRAM = None


def _get_program():
    global _PROGRAM
    if _PROGRAM is None:
        _PROGRAM = build_core_program()
    return _PROGRAM


def run_sharded(nc, in_maps, **kw):
    """run_bass_kernel_spmd with retries: the first execution on a freshly
    attached device occasionally dies with NRT_EXEC_UNIT_UNRECOVERABLE."""
    last = None
    for _ in range(3):
        try:
            return run_bass_kernel_spmd(nc, in_maps,
                                        core_ids=list(range(N_CORES)), **kw)
        except Exception as e:  # noqa: BLE001
            last = e
    raise last


def kernel(x, w_qkv, b_qkv, w_proj, b_proj):
    x = np.asarray(x, dtype=np.float32)
    w_qkv = np.asarray(w_qkv, dtype=np.float32)
    b_qkv = np.asarray(b_qkv, dtype=np.float32)
    w_proj = np.asarray(w_proj, dtype=np.float32)
    b_proj = np.asarray(b_proj, dtype=np.float32)

    nc = _get_program()
    in_maps = shard_inputs(x, w_qkv, b_qkv, w_proj)
    res = run_sharded(nc, in_maps)

    B, S, D = x.shape
    out = np.empty((B, S, D), dtype=np.float32)
    for b in range(B):
        out[b] = res.results[2 * b]["out"] + res.results[2 * b + 1]["out"] + b_proj
    return out
